# revision 58
# baseline (speedup 1.0000x reference)
"""Trainium2 Bass kernel for the nn_Aggregate GNN message-passing problem.

Computation (see reference):
    keep = (A > 0) limited to the first `neibor_num` set entries per row
    nb_mean = (keep @ X) / max(cnt, 1)
    out = leaky_relu(X @ W_line.T + b_line)
        + where(cnt > 0, leaky_relu(nb_mean @ W_nb.T + b_nb), 0)

Sharding: rows of A / output rows are split across 8 cores (1024 rows each).
No collectives are needed: each core gets its A row-block (transposed), its
X row-block (transposed), the shared X head rows, and the weights.

Key structural fact exploited: `keep` zeroes every set bit after the
`neibor_num`-th, so only the first C columns of A can contribute, where C
bounds the column position of the nn-th set bit over all rows.  The host
verifies exactly (cheaply) that every row reaches `neibor_num` set bits
within the first C=256 columns; in that case cnt == neibor_num for every
row and the kernel contracts over 256 neighbor candidates instead of 8192.
If the check fails (it cannot for the target input distribution), a numpy
fallback computes the exact reference semantics.

Device pipeline per core (rows R=1024, C=256, Cin=Cout=512):
  1. mask:    the host ships mbT[j, r] = (A[r, j] > 0) directly as fp8
              0/1 in transposed layout (exact, 1 byte/entry, and a valid
              PE operand dtype - no on-device convert needed).
  2. prefix:  cumT = LTRI.T @ mbT per 128-column chunk (+ ONES.T @ mbT of
              earlier chunks) gives the inclusive prefix count of set bits
              along the row, in transposed layout, on the PE (fp8 inputs,
              fp32 accumulation; counts <= 256 so exact).
  3. keepT = (cumT <= nn) * mbT                      (one fused DVE op)
  4. Xw = X_head @ (W_nb.T/nn) + 1 (x) (b_nb/nn)     (PE; no mask dep)
  5. xj = leaky(keep @ Xw)                           (PE + ACT Lrelu)
     xi = leaky(X_blk @ W_line.T + b_line)           (PE + ACT Lrelu)
     out = xi + xj                                   (DVE, fp32)
Step 4 uses associativity ((keep @ Xh) @ W == keep @ (Xh @ W)) to shrink
the neighbor stage from 48 matmuls to 24, and it depends only on weights,
so it fills the PE while the mask pipeline resolves.  Since cnt == nn on
the fast path, adding b_nb/nn to every row of Xw makes keep @ Xw land
exactly +b_nb - the per-tile xj bias matmuls collapse into 2 rank-1 terms.
The xj stage is emitted before the xi stage: the in-order PE queue must
not block ready xj work behind xi matmuls that wait on the delayed
xt/wlt bulk DMA.  xi biases ride k=1 ones-row matmuls in the PSUM group.

Precision: all matmuls accumulate fp32 in PSUM.  The mask/count side
(A-mask, LTRI/ONES) is exact fp8; the keep-mask and neighbor-mean path
(X_head, nbm, W_nb) is bf16 - masks are integer-exact and the
neighbor-mean path has small magnitudes, so its bf16 rounding
contributes little.  The precision-critical self-linear (X @ W_line.T,
operands at full scale) runs in fp16 (e5m10).  Measured absmax error vs
the fp32 reference: ~4e-4 of the output scale.

DMA strategy: each logical input is packed on the host into one wide
[128, *] (or [1, *]) tensor; the A-block load (which gates the whole PE
schedule) is split 4 ways across two SW-DGE queues so it lands on
parallel DMA engines, and the bulk stage-2 operands are held behind it
with explicit dependencies so they don't steal HBM bandwidth from the
critical path.
"""

import numpy as np

NCORES = 8
N = 8192
CIN = 512
COUT = 512
R = N // NCORES          # rows per core
C = 256                  # neighbor-candidate column window
KC = C // 128            # 128-col chunks of the window
MC = CIN // 128          # 128-row chunks of the feature dim
RT = R // 128            # 128-row output tiles per core
NEG_SLOPE = 0.01         # jax.nn.leaky_relu default

_nc_cache = {}
LAST_RESULT = None       # BassKernelResults of the most recent device run
SIM_SAFE = False         # CoreSim lacks Lrelu; True swaps in a Relu decomposition


def _build_nc(nn: int):
    import concourse.bass as bass
    import concourse.bacc as bacc
    import concourse.mybir as mybir
    import concourse.tile as tile
    from concourse.tile import add_dep_helper

    F32 = mybir.dt.float32
    BF16 = mybir.dt.bfloat16  # PE fast path for the mask/xj side
    FP16 = mybir.dt.float16   # e5m10 for the precision-critical xi path
    FP8 = mybir.dt.float8e4   # exact for the 0/1 mask; skips any convert op
    AF = mybir.ActivationFunctionType
    OP = mybir.AluOpType

    nc = bacc.Bacc("TRN2", target_bir_lowering=False, debug=False)

    at_d = nc.dram_tensor("at", [128, KC * R], FP8, kind="ExternalInput")
    xht_d = nc.dram_tensor("xht", [128, MC * C], BF16, kind="ExternalInput")
    xt_d = nc.dram_tensor("xt", [128, MC * R], FP16, kind="ExternalInput")
    wnbt_d = nc.dram_tensor("wnbt", [128, MC * COUT], BF16, kind="ExternalInput")
    wlt_d = nc.dram_tensor("wlt", [128, MC * COUT], FP16, kind="ExternalInput")
    sm_d = nc.dram_tensor("sm", [128, 256], FP8, kind="ExternalInput")
    rcb_d = nc.dram_tensor("rcb", [1, COUT + 128], BF16, kind="ExternalInput")
    rcf_d = nc.dram_tensor("rcf", [1, COUT + 128], FP16, kind="ExternalInput")
    out_d = nc.dram_tensor("out", [R, COUT], F32, kind="ExternalOutput")

    with tile.TileContext(nc) as tc:
        with (
            tc.tile_pool(name="const", bufs=1) as constp,
            tc.tile_pool(name="mask", bufs=1) as maskp,
            tc.tile_pool(name="work", bufs=3) as workp,
            tc.tile_pool(name="xjbuf", bufs=8) as xjp,
            tc.tile_pool(name="psum2", bufs=2, space=bass.MemorySpace.PSUM) as psump2,
            tc.tile_pool(name="psum3", bufs=3, space=bass.MemorySpace.PSUM) as psump3,
        ):
            # --- latency-critical loads (SW DGE: aggregates the 2KB lines
            # of these small tensors into 16KB packets) ------------------
            at_sb = maskp.tile([128, KC * R], FP8, name="at_sb")
            at_dmas = [(nc.gpsimd if p % 2 == 0 else nc.scalar).dma_start(
                           at_sb[p * 32:(p + 1) * 32, :],
                           at_d[p * 32:(p + 1) * 32, :])
                       for p in range(4)]
            sm = constp.tile([128, 256], FP8, name="sm_sb")
            nc.scalar.dma_start(sm[:], sm_d[:])
            xht_sb = constp.tile([128, MC * C], BF16, name="xht_sb")
            xh_half = MC * C // 2
            nc.scalar.dma_start(xht_sb[:, :xh_half], xht_d[:, :xh_half])
            nc.scalar.dma_start(xht_sb[:, xh_half:], xht_d[:, xh_half:])
            wnbt_sb = constp.tile([128, MC * COUT], BF16, name="wnbt_sb")
            wh = MC * COUT // 2
            nc.gpsimd.dma_start(wnbt_sb[:, :wh], wnbt_d[:, :wh])
            nc.gpsimd.dma_start(wnbt_sb[:, wh:], wnbt_d[:, wh:])
            rcb = constp.tile([1, COUT + 128], BF16, name="rcb_sb")
            nc.scalar.dma_start(rcb[:], rcb_d[:])
            rcf = constp.tile([1, COUT + 128], FP16, name="rcf_sb")
            nc.scalar.dma_start(rcf[:], rcf_d[:])

            ltri = sm[:, 0:128]
            ones = sm[:, 128:256]
            bnbs = rcb[:, 0:COUT]      # b_nb / nn (folded into Xw)
            onesb = rcb[:, COUT:]
            bl = rcf[:, 0:COUT]
            onesf = rcf[:, COUT:]
            at = [at_sb[:, t * R:(t + 1) * R] for t in range(KC)]
            wnbt = [wnbt_sb[:, m * COUT:(m + 1) * COUT] for m in range(MC)]

            # --- bulk stage-2 operands (HW DGE; 4-8KB lines).  Held back
            # behind the at transfer so the mask pipeline (which gates the
            # whole PE schedule) isn't starved of HBM bandwidth. ----------
            wlt_sb = constp.tile([128, MC * COUT], FP16, name="wlt_sb")
            d2 = nc.sync.dma_start(wlt_sb[:], wlt_d[:])
            xt_sb = constp.tile([128, MC * R], FP16, name="xt_sb")
            c1, c2 = 2 * CIN, 4 * CIN
            d1a = nc.sync.dma_start(xt_sb[:, :c1], xt_d[:, :c1])
            d1b = nc.sync.dma_start(xt_sb[:, c1:c2], xt_d[:, c1:c2])
            d1c = nc.sync.dma_start(xt_sb[:, c2:], xt_d[:, c2:])
            for b in (d2, d1a, d1b, d1c):
                for a in at_dmas:
                    add_dep_helper(b.ins, a.ins, sync=True,
                                   reason="bulk loads yield HBM to the mask path")
            # xt is packed per r-tile: block (r, m) at r*CIN + m*128
            wlt = [wlt_sb[:, m * COUT:(m + 1) * COUT] for m in range(MC)]

            # 1. the host ships (A > 0) directly as exact fp8 0/1
            mb = at

            # 2+3. prefix count along the row (PE) -> keep mask (DVE)
            keep = []
            for t in range(KC):
                keep_t = maskp.tile([128, R], BF16, name=f"keep{t}")
                keep.append(keep_t)
            # h-outer: both chunks' first row-halves (which gate the first
            # xj tiles) come off the DVE before the second halves
            for h in range(R // 512):
                for t in range(KC):
                    sl = slice(h * 512, (h + 1) * 512)
                    cum = psump3.tile([128, 512], F32, name="cum")
                    for s in range(t + 1):
                        nc.tensor.matmul(
                            cum[:],
                            ltri if s == t else ones,
                            mb[s][:, sl],
                            start=(s == 0),
                            stop=(s == t),
                        )
                    # keep = (cum <= nn) * mb.  The first row-half is split
                    # into quarters so the first xj tiles (which only read
                    # keep[:, 0:256]) unblock after a 256-wide DVE op.
                    if h == 0:
                        for q in range(2):
                            qs = slice(h * 512 + q * 256,
                                       h * 512 + (q + 1) * 256)
                            cs_q = slice(q * 256, (q + 1) * 256)
                            nc.vector.scalar_tensor_tensor(
                                keep[t][:, qs], cum[:, cs_q], float(nn),
                                mb[t][:, qs], op0=OP.is_le, op1=OP.mult,
                            )
                    else:
                        nc.vector.scalar_tensor_tensor(
                            keep[t][:, sl], cum[:], float(nn), mb[t][:, sl],
                            op0=OP.is_le, op1=OP.mult,
                        )

            # 4. Xw[cand, o] = X_head @ (W_nb.T/nn)  -- by associativity,
            #    (keep @ X_head) @ WnbT == keep @ (X_head @ WnbT).  Xw has
            #    no mask dependency, so these matmuls fill the PE while
            #    the mask pipeline resolves, and they shrink the xj stage
            #    from 16+32 matmuls to 8+16.
            xw = []
            for cc in range(KC):
                psxw = psump2.tile([128, COUT], F32, name="psxw")
                for m in range(MC):
                    nc.tensor.matmul(
                        psxw[:],
                        xht_sb[:, m * C + cc * 128:m * C + (cc + 1) * 128],
                        wnbt[m],
                        start=(m == 0),
                        stop=False,
                    )
                # rank-1 bias term: every row of Xw gains b_nb/nn, and
                # sum(keep) == nn per output row, so keep @ Xw lands the
                # exact +b_nb (8 per-tile bias matmuls -> these 2)
                nc.tensor.matmul(psxw[:], onesb, bnbs, start=False, stop=True)
                xw_cc = maskp.tile([128, COUT], BF16, name=f"xw{cc}")
                if cc == 0:
                    nc.scalar.activation(xw_cc[:], psxw[:], AF.Copy)
                else:
                    nc.vector.tensor_copy(xw_cc[:], psxw[:])
                xw.append(xw_cc)

            # 5. two linears + leaky relu + add, per 128-row output tile
            def leaky(ps_ap, out_ap):
                # takes APs (tile slices)
                if SIM_SAFE:
                    fd = ps_ap.shape[-1]
                    t = workp.tile([128, COUT], F32, name="lrt")
                    nc.scalar.activation(t[:, :fd], ps_ap, AF.Relu,
                                         scale=1.0 - NEG_SLOPE)
                    nc.vector.scalar_tensor_tensor(
                        out_ap, ps_ap, NEG_SLOPE, t[:, :fd],
                        op0=OP.mult, op1=OP.add)
                else:
                    nc.scalar.activation(out_ap, ps_ap, AF.Lrelu,
                                         alpha=NEG_SLOPE)

            # xj phase first: its operands (keep, Xw) are ready before the
            # delayed xt/wlt bulk lands, and the PE queue is in-order -- an
            # early xi stall must not block ready xj work.  xj results park
            # in an 8-deep buffer until the xi phase's adds consume them.
            xjs = []
            for r in range(RT):
                rsl = slice(r * 128, (r + 1) * 128)
                psj = psump3.tile([128, COUT], F32, name="psj", tag="ps2")
                for cc in range(KC):
                    nc.tensor.matmul(
                        psj[:], keep[cc][:, rsl], xw[cc][:],
                        start=(cc == 0), stop=(cc == KC - 1),
                    )
                xj = xjp.tile([128, COUT], F32, name="xj", tag="xj")
                leaky(psj[:], xj[:])
                xjs.append(xj)

            for r in range(RT):
                rsl = slice(r * 128, (r + 1) * 128)
                xj = xjs[r]
                psi = psump3.tile([128, COUT], F32, name="psi", tag="ps2")
                for m in range(MC):
                    nc.tensor.matmul(
                        psi[:],
                        xt_sb[:, r * CIN + m * 128:r * CIN + (m + 1) * 128],
                        wlt[m],
                        start=(m == 0), stop=False,
                    )
                nc.tensor.matmul(psi[:], onesf, bl, start=False, stop=True)
                xi = workp.tile([128, COUT], F32, name="xi")
                ot = workp.tile([128, COUT], F32, name="ot")
                eng = nc.sync if r % 2 == 0 else nc.gpsimd
                if r == RT - 1:
                    # split the entire trailing chain (leaky->add->store) of
                    # the final tile into column halves: the h1 leaky runs on
                    # ACT while h0's add/store already drain on DVE/DMA
                    for hh in range(2):
                        cs = slice(hh * (COUT // 2), (hh + 1) * (COUT // 2))
                        leaky(psi[:, cs], xi[:, cs])
                        nc.vector.tensor_tensor(ot[:, cs], xi[:, cs], xj[:, cs],
                                                op=OP.add)
                        (nc.sync if hh == 0 else nc.gpsimd).dma_start(
                            out_d[rsl, cs], ot[:, cs])
                else:
                    leaky(psi[:], xi[:])
                    nc.vector.tensor_tensor(ot[:], xi[:], xj[:], op=OP.add)
                    eng.dma_start(out_d[rsl, :], ot[:])

    nc.compile()
    return nc


def _get_nc(nn: int):
    if nn not in _nc_cache:
        _nc_cache[nn] = _build_nc(nn)
    return _nc_cache[nn]


def _numpy_fallback(X, A, W_nb, b_nb, W_line, b_line, nn):
    def leaky(x):
        return np.where(x >= 0, x, NEG_SLOPE * x)

    Ab = A > 0
    keep = Ab & (np.cumsum(Ab.astype(np.int64), axis=1) <= nn)
    cnt = keep.sum(axis=1, keepdims=True).astype(X.dtype)
    nb_sum = keep.astype(X.dtype) @ X
    nb_mean = nb_sum / np.maximum(cnt, 1.0)
    xj = leaky(nb_mean @ W_nb.T + b_nb)
    xi = leaky(X @ W_line.T + b_line)
    return (xi + np.where(cnt > 0, xj, 0.0)).astype(np.float32)


def _pack_rtile(arr):
    """[MC*128, RT*128] -> [128, RT*MC*128]: block (r, m) at r*CIN + m*128."""
    f, rr = arr.shape
    return np.ascontiguousarray(
        arr.reshape(f // 128, 128, rr // 128, 128)
           .transpose(1, 2, 0, 3).reshape(128, -1))


def _pack128(arr):
    """[128*k, m] -> [128, k*m] with block i in columns [i*m:(i+1)*m]."""
    k = arr.shape[0] // 128
    return np.ascontiguousarray(
        arr.reshape(k, 128, arr.shape[1]).transpose(1, 0, 2).reshape(128, -1))


def build_in_maps(X, A, W_nb, b_nb, W_line, b_line, nn):
    """Shard the full inputs into one input map per core."""
    import ml_dtypes
    bf = ml_dtypes.bfloat16
    f8 = ml_dtypes.float8_e4m3
    ATall = np.ascontiguousarray((A[:, :C] > 0).T.astype(f8))        # [C, N]
    XTall = np.ascontiguousarray(X.T.astype(np.float16))            # [CIN, N]
    xht = _pack128(np.ascontiguousarray(X[:C, :].T).astype(bf))      # [128, MC*C]
    wnbt = _pack128(np.ascontiguousarray(W_nb.T.astype(np.float32)
                                         * np.float32(1.0 / nn)).astype(bf))
    wlt = _pack128(np.ascontiguousarray(W_line.T.astype(np.float16)))
    sm = np.concatenate([np.triu(np.ones((128, 128), f8)),
                         np.ones((128, 128), f8)], axis=1)   # [128, 256]
    rcb = np.concatenate([(b_nb.astype(np.float32)
                           * np.float32(1.0 / nn)).astype(bf).reshape(1, COUT),
                          np.ones((1, 128), bf)], axis=1)
    rcf = np.concatenate([b_line.astype(np.float16).reshape(1, COUT),
                          np.ones((1, 128), np.float16)], axis=1)
    in_maps = []
    for c in range(NCORES):
        rows = slice(c * R, (c + 1) * R)
        in_maps.append({
            "at": _pack128(ATall[:, rows]),
            "xht": xht,
            "xt": _pack_rtile(XTall[:, rows]),
            "wnbt": wnbt,
            "wlt": wlt,
            "sm": sm,
            "rcb": rcb,
            "rcf": rcf,
        })
    return in_maps


def kernel(**inputs) -> np.ndarray:
    global LAST_RESULT
    X = np.ascontiguousarray(np.asarray(inputs["X"], dtype=np.float32))
    A = np.ascontiguousarray(np.asarray(inputs["A"], dtype=np.int32))
    W_nb = np.asarray(inputs["W_nb"], dtype=np.float32)
    b_nb = np.asarray(inputs["b_nb"], dtype=np.float32)
    W_line = np.asarray(inputs["W_line"], dtype=np.float32)
    b_line = np.asarray(inputs["b_line"], dtype=np.float32)
    nn = int(np.asarray(inputs["neibor_num"]))

    # Fast path requires: every row reaches nn set bits within the first C
    # columns (=> keep-mask confined to [:, :C] and cnt == nn > 0 per row).
    fast = (
        X.shape == (N, CIN) and A.shape == (N, N) and 1 <= nn <= C
        and int(np.count_nonzero(A[:, :C] > 0, axis=1).min()) >= nn
    )
    if not fast:
        return _numpy_fallback(X, A, W_nb, b_nb, W_line, b_line, nn)

    import os

    in_maps = build_in_maps(X, A, W_nb, b_nb, W_line, b_line, nn)
    nc = _get_nc(nn)
    if os.environ.get("BASS_TRACE"):
        from concourse.bass_utils import run_bass_kernel_spmd
        res = run_bass_kernel_spmd(nc, in_maps, core_ids=list(range(NCORES)))
        LAST_RESULT = res
        return np.concatenate([r["out"] for r in res.results], axis=0)
    outs = _run_cached(nc, nn, in_maps)
    return np.concatenate(outs, axis=0)


_runner_cache = {}


def _run_cached(nc, nn, in_maps):
    """Execute the compiled program on the 8 cores, caching the jitted
    executable across calls (mirrors bass2jax.run_bass_via_pjrt's
    multi-core path; falls back to it on any setup error)."""
    import jax
    import concourse.mybir as mybir
    from concourse import bass2jax

    if nn not in _runner_cache:
        try:
            bass2jax.install_neuronx_cc_hook()
            part_name = (nc.partition_id_tensor.name
                         if nc.partition_id_tensor else None)
            in_names, out_names, out_avals, zero_shapes = [], [], [], []
            for alloc in nc.m.functions[0].allocations:
                if not isinstance(alloc, mybir.MemoryLocationSet):
                    continue
                name = alloc.memorylocations[0].name
                if alloc.kind == "ExternalInput":
                    if name != part_name:
                        in_names.append(name)
                elif alloc.kind == "ExternalOutput":
                    out_names.append(name)
                    np_dt = mybir.dt.np(alloc.dtype)
                    out_avals.append(jax.core.ShapedArray(
                        tuple(alloc.tensor_shape), np_dt))
                    zero_shapes.append((tuple(alloc.tensor_shape), np_dt))
            n_params = len(in_names)
            all_names = tuple(in_names + out_names
                              + ([part_name] if part_name else []))

            def _body(*args):
                operands = list(args)
                if part_name:
                    operands.append(bass2jax.partition_id_tensor())
                outs = bass2jax._bass_exec_p.bind(
                    *operands,
                    out_avals=tuple(out_avals),
                    in_names=all_names,
                    out_names=tuple(out_names),
                    lowering_input_output_aliases=(),
                    sim_require_finite=True,
                    sim_require_nnan=True,
                    nc=nc,
                )
                return tuple(outs)

            from jax.sharding import Mesh, PartitionSpec
            try:
                from jax.experimental.shard_map import shard_map
            except ImportError:
                from jax.shard_map import shard_map
            devices = jax.devices()[:NCORES]
            assert len(devices) == NCORES
            mesh = Mesh(np.asarray(devices), ("core",))
            n_outs = len(out_names)
            sharded = jax.jit(
                shard_map(_body, mesh=mesh,
                          in_specs=(PartitionSpec("core"),) * (n_params + n_outs),
                          out_specs=(PartitionSpec("core"),) * n_outs,
                          check_rep=False),
                donate_argnums=tuple(range(n_params, n_params + n_outs)),
                keep_unused=True,
            )
            _runner_cache[nn] = (sharded, in_names, out_names, zero_shapes)
        except Exception:
            _runner_cache[nn] = None
    cached = _runner_cache[nn]
    if cached is None:
        from concourse.bass_utils import run_bass_kernel_spmd
        res = run_bass_kernel_spmd(nc, in_maps, core_ids=list(range(NCORES)))
        return [r["out"] for r in res.results]
    sharded, in_names, out_names, zero_shapes = cached
    concat_in = [np.concatenate([np.asarray(m[name]) for m in in_maps], axis=0)
                 for name in in_names]
    concat_zeros = [np.zeros((NCORES * sh[0],) + sh[1:], dt)
                    for sh, dt in zero_shapes]
    out_arrs = sharded(*concat_in, *concat_zeros)
    oi = out_names.index("out")
    full = np.asarray(out_arrs[oi]).reshape(NCORES, R, COUT)
    return [full[c] for c in range(NCORES)]


if __name__ == "__main__":
    rng = np.random.default_rng(0)
    X = rng.standard_normal((N, CIN), dtype=np.float32)
    A = (rng.random((N, N)) < 0.5).astype(np.int32)
    W_nb = rng.standard_normal((COUT, CIN), dtype=np.float32) * 0.04
    b_nb = rng.standard_normal(COUT, dtype=np.float32) * 0.04
    W_line = rng.standard_normal((COUT, CIN), dtype=np.float32) * 0.04
    b_line = rng.standard_normal(COUT, dtype=np.float32) * 0.04
    out = kernel(X=X, A=A, W_nb=W_nb, b_nb=b_nb, W_line=W_line,
                 b_line=b_line, neibor_num=64)
    exp = _numpy_fallback(X, A, W_nb, b_nb, W_line, b_line, 64)
    err = np.abs(out - exp).max() / np.abs(exp).max()
    print("self-test rel err:", err)


# revision 59
# speedup vs baseline: 1.0960x; 1.0960x over previous
"""Trainium2 Bass kernel for the nn_Aggregate GNN message-passing problem.

Computation (see reference):
    keep = (A > 0) limited to the first `neibor_num` set entries per row
    nb_mean = (keep @ X) / max(cnt, 1)
    out = leaky_relu(X @ W_line.T + b_line)
        + where(cnt > 0, leaky_relu(nb_mean @ W_nb.T + b_nb), 0)

Sharding: rows of A / output rows are split across 8 cores (1024 rows each).
No collectives are needed: each core gets its A row-block (transposed), its
X row-block (transposed), the shared X head rows, and the weights.

Key structural fact exploited: `keep` zeroes every set bit after the
`neibor_num`-th, so only the first C columns of A can contribute, where C
bounds the column position of the nn-th set bit over all rows.  The host
verifies exactly (cheaply) that every row reaches `neibor_num` set bits
within the first C=256 columns; in that case cnt == neibor_num for every
row and the kernel contracts over 256 neighbor candidates instead of 8192.
If the check fails (it cannot for the target input distribution), a numpy
fallback computes the exact reference semantics.

Device pipeline per core (rows R=1024, C=256, Cin=Cout=512):
  1. mask:    the host ships mbT[j, r] = (A[r, j] > 0) directly as fp8
              0/1 in transposed layout (exact, 1 byte/entry, and a valid
              PE operand dtype - no on-device convert needed).
  2. prefix:  cumT = LTRI.T @ mbT per 128-column chunk (+ ONES.T @ mbT of
              earlier chunks) gives the inclusive prefix count of set bits
              along the row, in transposed layout, on the PE (fp8 inputs,
              fp32 accumulation; counts <= 256 so exact).
  3. keepT = (cumT <= nn) * mbT                      (one fused DVE op)
  4. Xw = X_head @ (W_nb.T/nn) + 1 (x) (b_nb/nn)     (PE; no mask dep)
  5. xj = leaky(keep @ Xw)                           (PE + ACT Lrelu)
     xi = leaky(X_blk @ W_line.T + b_line)           (PE + ACT Lrelu)
     out = xi + xj                                   (DVE, fp32)
Step 4 uses associativity ((keep @ Xh) @ W == keep @ (Xh @ W)) to shrink
the neighbor stage from 48 matmuls to 24, and it depends only on weights,
so it fills the PE while the mask pipeline resolves.  Since cnt == nn on
the fast path, adding b_nb/nn to every row of Xw makes keep @ Xw land
exactly +b_nb - the per-tile xj bias matmuls collapse into 2 rank-1 terms.
The xj stage is emitted before the xi stage: the in-order PE queue must
not block ready xj work behind xi matmuls that wait on the delayed
xt/wlt bulk DMA.  xi biases ride k=1 ones-row matmuls in the PSUM group.

Precision: all matmuls accumulate fp32 in PSUM.  The mask/count side
(A-mask, LTRI/ONES) is exact fp8; the keep-mask and neighbor-mean path
(X_head, nbm, W_nb) is bf16 - masks are integer-exact and the
neighbor-mean path has small magnitudes, so its bf16 rounding
contributes little.  The precision-critical self-linear (X @ W_line.T,
operands at full scale) runs in fp16 (e5m10).  Measured absmax error vs
the fp32 reference: ~4e-4 of the output scale.

DMA strategy: each logical input is packed on the host into one wide
[128, *] (or [1, *]) tensor; the A-block load (which gates the whole PE
schedule) is split 4 ways across two SW-DGE queues so it lands on
parallel DMA engines, and the bulk stage-2 operands are held behind it
with explicit dependencies so they don't steal HBM bandwidth from the
critical path.
"""

import numpy as np

NCORES = 8
N = 8192
CIN = 512
COUT = 512
R = N // NCORES          # rows per core
C = 256                  # neighbor-candidate column window
KC = C // 128            # 128-col chunks of the window
MC = CIN // 128          # 128-row chunks of the feature dim
RT = R // 128            # 128-row output tiles per core
NEG_SLOPE = 0.01         # jax.nn.leaky_relu default

_nc_cache = {}
LAST_RESULT = None       # BassKernelResults of the most recent device run
SIM_SAFE = False         # CoreSim lacks Lrelu; True swaps in a Relu decomposition


def _build_nc(nn: int):
    import concourse.bass as bass
    import concourse.bacc as bacc
    import concourse.mybir as mybir
    import concourse.tile as tile
    from concourse.tile import add_dep_helper

    F32 = mybir.dt.float32
    BF16 = mybir.dt.bfloat16  # PE fast path for the mask/xj side
    FP16 = mybir.dt.float16   # e5m10 for the precision-critical xi path
    FP8 = mybir.dt.float8e4   # exact for the 0/1 mask; skips any convert op
    AF = mybir.ActivationFunctionType
    OP = mybir.AluOpType

    nc = bacc.Bacc("TRN2", target_bir_lowering=False, debug=False)

    at_d = nc.dram_tensor("at", [128, KC * R], FP8, kind="ExternalInput")
    xht_d = nc.dram_tensor("xht", [128, MC * C], BF16, kind="ExternalInput")
    xt_d = nc.dram_tensor("xt", [128, MC * R], FP16, kind="ExternalInput")
    wnbt_d = nc.dram_tensor("wnbt", [128, MC * COUT], BF16, kind="ExternalInput")
    wlt_d = nc.dram_tensor("wlt", [128, MC * COUT], FP16, kind="ExternalInput")
    sm_d = nc.dram_tensor("sm", [128, 256], FP8, kind="ExternalInput")
    rcb_d = nc.dram_tensor("rcb", [1, COUT + 128], BF16, kind="ExternalInput")
    rcf_d = nc.dram_tensor("rcf", [1, COUT + 128], FP16, kind="ExternalInput")
    out_d = nc.dram_tensor("out", [R, COUT], F32, kind="ExternalOutput")

    with tile.TileContext(nc) as tc:
        with (
            tc.tile_pool(name="const", bufs=1) as constp,
            tc.tile_pool(name="mask", bufs=1) as maskp,
            tc.tile_pool(name="work", bufs=3) as workp,
            tc.tile_pool(name="xjbuf", bufs=8) as xjp,
            tc.tile_pool(name="psum2", bufs=2, space=bass.MemorySpace.PSUM) as psump2,
            tc.tile_pool(name="psum3", bufs=3, space=bass.MemorySpace.PSUM) as psump3,
        ):
            # --- latency-critical loads (SW DGE: aggregates the 2KB lines
            # of these small tensors into 16KB packets) ------------------
            at_sb = maskp.tile([128, KC * R], FP8, name="at_sb")
            at_dmas = [(nc.gpsimd if p % 2 == 0 else nc.scalar).dma_start(
                           at_sb[p * 32:(p + 1) * 32, :],
                           at_d[p * 32:(p + 1) * 32, :])
                       for p in range(4)]
            sm = constp.tile([128, 256], FP8, name="sm_sb")
            nc.scalar.dma_start(sm[:], sm_d[:])
            xht_sb = constp.tile([128, MC * C], BF16, name="xht_sb")
            xh_half = MC * C // 2
            nc.scalar.dma_start(xht_sb[:, :xh_half], xht_d[:, :xh_half])
            nc.scalar.dma_start(xht_sb[:, xh_half:], xht_d[:, xh_half:])
            wnbt_sb = constp.tile([128, MC * COUT], BF16, name="wnbt_sb")
            wh = MC * COUT // 2
            nc.gpsimd.dma_start(wnbt_sb[:, :wh], wnbt_d[:, :wh])
            nc.gpsimd.dma_start(wnbt_sb[:, wh:], wnbt_d[:, wh:])
            rcb = constp.tile([1, COUT + 128], BF16, name="rcb_sb")
            nc.scalar.dma_start(rcb[:], rcb_d[:])
            rcf = constp.tile([1, COUT + 128], FP16, name="rcf_sb")
            nc.scalar.dma_start(rcf[:], rcf_d[:])

            ltri = sm[:, 0:128]
            ones = sm[:, 128:256]
            bnbs = rcb[:, 0:COUT]      # b_nb / nn (folded into Xw)
            onesb = rcb[:, COUT:]
            bl = rcf[:, 0:COUT]
            onesf = rcf[:, COUT:]
            at = [at_sb[:, t * R:(t + 1) * R] for t in range(KC)]
            wnbt = [wnbt_sb[:, m * COUT:(m + 1) * COUT] for m in range(MC)]

            # --- bulk stage-2 operands (HW DGE; 4-8KB lines).  Held back
            # behind the at transfer so the mask pipeline (which gates the
            # whole PE schedule) isn't starved of HBM bandwidth. ----------
            wlt_sb = constp.tile([128, MC * COUT], FP16, name="wlt_sb")
            d2 = nc.sync.dma_start(wlt_sb[:], wlt_d[:])
            xt_sb = constp.tile([128, MC * R], FP16, name="xt_sb")
            c1, c2 = 2 * CIN, 4 * CIN
            d1a = nc.sync.dma_start(xt_sb[:, :c1], xt_d[:, :c1])
            d1b = nc.sync.dma_start(xt_sb[:, c1:c2], xt_d[:, c1:c2])
            d1c = nc.sync.dma_start(xt_sb[:, c2:], xt_d[:, c2:])
            for b in (d2, d1a, d1b, d1c):
                for a in at_dmas:
                    add_dep_helper(b.ins, a.ins, sync=True,
                                   reason="bulk loads yield HBM to the mask path")
            # xt is packed per r-tile: block (r, m) at r*CIN + m*128
            wlt = [wlt_sb[:, m * COUT:(m + 1) * COUT] for m in range(MC)]

            # 1. the host ships (A > 0) directly as exact fp8 0/1
            mb = at

            # 2+3. prefix count along the row (PE) -> keep mask (DVE)
            keep = []
            for t in range(KC):
                keep_t = maskp.tile([128, R], BF16, name=f"keep{t}")
                keep.append(keep_t)
            # h-outer: both chunks' first row-halves (which gate the first
            # xj tiles) come off the DVE before the second halves
            for h in range(R // 512):
                for t in range(KC):
                    sl = slice(h * 512, (h + 1) * 512)
                    cum = psump3.tile([128, 512], F32, name="cum")
                    for s in range(t + 1):
                        nc.tensor.matmul(
                            cum[:],
                            ltri if s == t else ones,
                            mb[s][:, sl],
                            start=(s == 0),
                            stop=(s == t),
                        )
                    # keep = (cum <= nn) * mb
                    nc.vector.scalar_tensor_tensor(
                        keep[t][:, sl], cum[:], float(nn), mb[t][:, sl],
                        op0=OP.is_le, op1=OP.mult,
                    )

            # 4. Xw[cand, o] = X_head @ (W_nb.T/nn)  -- by associativity,
            #    (keep @ X_head) @ WnbT == keep @ (X_head @ WnbT).  Xw has
            #    no mask dependency, so these matmuls fill the PE while
            #    the mask pipeline resolves, and they shrink the xj stage
            #    from 16+32 matmuls to 8+16.
            xw = []
            for cc in range(KC):
                psxw = psump2.tile([128, COUT], F32, name="psxw")
                for m in range(MC):
                    nc.tensor.matmul(
                        psxw[:],
                        xht_sb[:, m * C + cc * 128:m * C + (cc + 1) * 128],
                        wnbt[m],
                        start=(m == 0),
                        stop=False,
                    )
                # rank-1 bias term: every row of Xw gains b_nb/nn, and
                # sum(keep) == nn per output row, so keep @ Xw lands the
                # exact +b_nb (8 per-tile bias matmuls -> these 2)
                nc.tensor.matmul(psxw[:], onesb, bnbs, start=False, stop=True)
                xw_cc = maskp.tile([128, COUT], BF16, name=f"xw{cc}")
                if cc == 0:
                    nc.scalar.activation(xw_cc[:], psxw[:], AF.Copy)
                else:
                    nc.vector.tensor_copy(xw_cc[:], psxw[:])
                xw.append(xw_cc)

            # 5. two linears + leaky relu + add, per 128-row output tile
            def leaky(ps_ap, out_ap):
                # takes APs (tile slices)
                if SIM_SAFE:
                    fd = ps_ap.shape[-1]
                    t = workp.tile([128, COUT], F32, name="lrt")
                    nc.scalar.activation(t[:, :fd], ps_ap, AF.Relu,
                                         scale=1.0 - NEG_SLOPE)
                    nc.vector.scalar_tensor_tensor(
                        out_ap, ps_ap, NEG_SLOPE, t[:, :fd],
                        op0=OP.mult, op1=OP.add)
                else:
                    nc.scalar.activation(out_ap, ps_ap, AF.Lrelu,
                                         alpha=NEG_SLOPE)

            # xj phase first: its operands (keep, Xw) are ready before the
            # delayed xt/wlt bulk lands, and the PE queue is in-order -- an
            # early xi stall must not block ready xj work.  xj results park
            # in an 8-deep buffer until the xi phase's adds consume them.
            xjs = []
            for r in range(RT):
                rsl = slice(r * 128, (r + 1) * 128)
                psj = psump3.tile([128, COUT], F32, name="psj", tag="ps2")
                for cc in range(KC):
                    nc.tensor.matmul(
                        psj[:], keep[cc][:, rsl], xw[cc][:],
                        start=(cc == 0), stop=(cc == KC - 1),
                    )
                xj = xjp.tile([128, COUT], F32, name="xj", tag="xj")
                leaky(psj[:], xj[:])
                xjs.append(xj)

            for r in range(RT):
                rsl = slice(r * 128, (r + 1) * 128)
                xj = xjs[r]
                psi = psump3.tile([128, COUT], F32, name="psi", tag="ps2")
                for m in range(MC):
                    nc.tensor.matmul(
                        psi[:],
                        xt_sb[:, r * CIN + m * 128:r * CIN + (m + 1) * 128],
                        wlt[m],
                        start=(m == 0), stop=False,
                    )
                nc.tensor.matmul(psi[:], onesf, bl, start=False, stop=True)
                xi = workp.tile([128, COUT], F32, name="xi")
                ot = workp.tile([128, COUT], F32, name="ot")
                eng = nc.sync if r % 2 == 0 else nc.gpsimd
                if r == RT - 1:
                    # split the entire trailing chain (leaky->add->store) of
                    # the final tile into column halves: the h1 leaky runs on
                    # ACT while h0's add/store already drain on DVE/DMA
                    for hh in range(2):
                        cs = slice(hh * (COUT // 2), (hh + 1) * (COUT // 2))
                        leaky(psi[:, cs], xi[:, cs])
                        nc.vector.tensor_tensor(ot[:, cs], xi[:, cs], xj[:, cs],
                                                op=OP.add)
                        (nc.sync if hh == 0 else nc.gpsimd).dma_start(
                            out_d[rsl, cs], ot[:, cs])
                else:
                    leaky(psi[:], xi[:])
                    nc.vector.tensor_tensor(ot[:], xi[:], xj[:], op=OP.add)
                    eng.dma_start(out_d[rsl, :], ot[:])

    nc.compile()
    return nc


def _get_nc(nn: int):
    if nn not in _nc_cache:
        _nc_cache[nn] = _build_nc(nn)
    return _nc_cache[nn]


def _numpy_fallback(X, A, W_nb, b_nb, W_line, b_line, nn):
    def leaky(x):
        return np.where(x >= 0, x, NEG_SLOPE * x)

    Ab = A > 0
    keep = Ab & (np.cumsum(Ab.astype(np.int64), axis=1) <= nn)
    cnt = keep.sum(axis=1, keepdims=True).astype(X.dtype)
    nb_sum = keep.astype(X.dtype) @ X
    nb_mean = nb_sum / np.maximum(cnt, 1.0)
    xj = leaky(nb_mean @ W_nb.T + b_nb)
    xi = leaky(X @ W_line.T + b_line)
    return (xi + np.where(cnt > 0, xj, 0.0)).astype(np.float32)


def _pack_rtile(arr):
    """[MC*128, RT*128] -> [128, RT*MC*128]: block (r, m) at r*CIN + m*128."""
    f, rr = arr.shape
    return np.ascontiguousarray(
        arr.reshape(f // 128, 128, rr // 128, 128)
           .transpose(1, 2, 0, 3).reshape(128, -1))


def _pack128(arr):
    """[128*k, m] -> [128, k*m] with block i in columns [i*m:(i+1)*m]."""
    k = arr.shape[0] // 128
    return np.ascontiguousarray(
        arr.reshape(k, 128, arr.shape[1]).transpose(1, 0, 2).reshape(128, -1))


def build_in_maps(X, A, W_nb, b_nb, W_line, b_line, nn):
    """Shard the full inputs into one input map per core."""
    import ml_dtypes
    bf = ml_dtypes.bfloat16
    f8 = ml_dtypes.float8_e4m3
    ATall = np.ascontiguousarray((A[:, :C] > 0).T.astype(f8))        # [C, N]
    XTall = np.ascontiguousarray(X.T.astype(np.float16))            # [CIN, N]
    xht = _pack128(np.ascontiguousarray(X[:C, :].T).astype(bf))      # [128, MC*C]
    wnbt = _pack128(np.ascontiguousarray(W_nb.T.astype(np.float32)
                                         * np.float32(1.0 / nn)).astype(bf))
    wlt = _pack128(np.ascontiguousarray(W_line.T.astype(np.float16)))
    sm = np.concatenate([np.triu(np.ones((128, 128), f8)),
                         np.ones((128, 128), f8)], axis=1)   # [128, 256]
    rcb = np.concatenate([(b_nb.astype(np.float32)
                           * np.float32(1.0 / nn)).astype(bf).reshape(1, COUT),
                          np.ones((1, 128), bf)], axis=1)
    rcf = np.concatenate([b_line.astype(np.float16).reshape(1, COUT),
                          np.ones((1, 128), np.float16)], axis=1)
    in_maps = []
    for c in range(NCORES):
        rows = slice(c * R, (c + 1) * R)
        in_maps.append({
            "at": _pack128(ATall[:, rows]),
            "xht": xht,
            "xt": _pack_rtile(XTall[:, rows]),
            "wnbt": wnbt,
            "wlt": wlt,
            "sm": sm,
            "rcb": rcb,
            "rcf": rcf,
        })
    return in_maps


def kernel(**inputs) -> np.ndarray:
    global LAST_RESULT
    X = np.ascontiguousarray(np.asarray(inputs["X"], dtype=np.float32))
    A = np.ascontiguousarray(np.asarray(inputs["A"], dtype=np.int32))
    W_nb = np.asarray(inputs["W_nb"], dtype=np.float32)
    b_nb = np.asarray(inputs["b_nb"], dtype=np.float32)
    W_line = np.asarray(inputs["W_line"], dtype=np.float32)
    b_line = np.asarray(inputs["b_line"], dtype=np.float32)
    nn = int(np.asarray(inputs["neibor_num"]))

    # Fast path requires: every row reaches nn set bits within the first C
    # columns (=> keep-mask confined to [:, :C] and cnt == nn > 0 per row).
    fast = (
        X.shape == (N, CIN) and A.shape == (N, N) and 1 <= nn <= C
        and int(np.count_nonzero(A[:, :C] > 0, axis=1).min()) >= nn
    )
    if not fast:
        return _numpy_fallback(X, A, W_nb, b_nb, W_line, b_line, nn)

    import os

    in_maps = build_in_maps(X, A, W_nb, b_nb, W_line, b_line, nn)
    nc = _get_nc(nn)
    if os.environ.get("BASS_TRACE"):
        from concourse.bass_utils import run_bass_kernel_spmd
        res = run_bass_kernel_spmd(nc, in_maps, core_ids=list(range(NCORES)))
        LAST_RESULT = res
        return np.concatenate([r["out"] for r in res.results], axis=0)
    outs = _run_cached(nc, nn, in_maps)
    return np.concatenate(outs, axis=0)


_runner_cache = {}


def _run_cached(nc, nn, in_maps):
    """Execute the compiled program on the 8 cores, caching the jitted
    executable across calls (mirrors bass2jax.run_bass_via_pjrt's
    multi-core path; falls back to it on any setup error)."""
    import jax
    import concourse.mybir as mybir
    from concourse import bass2jax

    if nn not in _runner_cache:
        try:
            bass2jax.install_neuronx_cc_hook()
            part_name = (nc.partition_id_tensor.name
                         if nc.partition_id_tensor else None)
            in_names, out_names, out_avals, zero_shapes = [], [], [], []
            for alloc in nc.m.functions[0].allocations:
                if not isinstance(alloc, mybir.MemoryLocationSet):
                    continue
                name = alloc.memorylocations[0].name
                if alloc.kind == "ExternalInput":
                    if name != part_name:
                        in_names.append(name)
                elif alloc.kind == "ExternalOutput":
                    out_names.append(name)
                    np_dt = mybir.dt.np(alloc.dtype)
                    out_avals.append(jax.core.ShapedArray(
                        tuple(alloc.tensor_shape), np_dt))
                    zero_shapes.append((tuple(alloc.tensor_shape), np_dt))
            n_params = len(in_names)
            all_names = tuple(in_names + out_names
                              + ([part_name] if part_name else []))

            def _body(*args):
                operands = list(args)
                if part_name:
                    operands.append(bass2jax.partition_id_tensor())
                outs = bass2jax._bass_exec_p.bind(
                    *operands,
                    out_avals=tuple(out_avals),
                    in_names=all_names,
                    out_names=tuple(out_names),
                    lowering_input_output_aliases=(),
                    sim_require_finite=True,
                    sim_require_nnan=True,
                    nc=nc,
                )
                return tuple(outs)

            from jax.sharding import Mesh, PartitionSpec
            try:
                from jax.experimental.shard_map import shard_map
            except ImportError:
                from jax.shard_map import shard_map
            devices = jax.devices()[:NCORES]
            assert len(devices) == NCORES
            mesh = Mesh(np.asarray(devices), ("core",))
            n_outs = len(out_names)
            sharded = jax.jit(
                shard_map(_body, mesh=mesh,
                          in_specs=(PartitionSpec("core"),) * (n_params + n_outs),
                          out_specs=(PartitionSpec("core"),) * n_outs,
                          check_rep=False),
                donate_argnums=tuple(range(n_params, n_params + n_outs)),
                keep_unused=True,
            )
            _runner_cache[nn] = (sharded, in_names, out_names, zero_shapes)
        except Exception:
            _runner_cache[nn] = None
    cached = _runner_cache[nn]
    if cached is None:
        from concourse.bass_utils import run_bass_kernel_spmd
        res = run_bass_kernel_spmd(nc, in_maps, core_ids=list(range(NCORES)))
        return [r["out"] for r in res.results]
    sharded, in_names, out_names, zero_shapes = cached
    concat_in = [np.concatenate([np.asarray(m[name]) for m in in_maps], axis=0)
                 for name in in_names]
    concat_zeros = [np.zeros((NCORES * sh[0],) + sh[1:], dt)
                    for sh, dt in zero_shapes]
    out_arrs = sharded(*concat_in, *concat_zeros)
    oi = out_names.index("out")
    full = np.asarray(out_arrs[oi]).reshape(NCORES, R, COUT)
    return [full[c] for c in range(NCORES)]


if __name__ == "__main__":
    rng = np.random.default_rng(0)
    X = rng.standard_normal((N, CIN), dtype=np.float32)
    A = (rng.random((N, N)) < 0.5).astype(np.int32)
    W_nb = rng.standard_normal((COUT, CIN), dtype=np.float32) * 0.04
    b_nb = rng.standard_normal(COUT, dtype=np.float32) * 0.04
    W_line = rng.standard_normal((COUT, CIN), dtype=np.float32) * 0.04
    b_line = rng.standard_normal(COUT, dtype=np.float32) * 0.04
    out = kernel(X=X, A=A, W_nb=W_nb, b_nb=b_nb, W_line=W_line,
                 b_line=b_line, neibor_num=64)
    exp = _numpy_fallback(X, A, W_nb, b_nb, W_line, b_line, 64)
    err = np.abs(out - exp).max() / np.abs(exp).max()
    print("self-test rel err:", err)
